# revision 1
# baseline (speedup 1.0000x reference)
"""Trainium2 Bass kernel for nn_Attention_78151224918608.

Dense transformer attention block: QKV proj + RoPE + GQA causal attention
+ output proj. Sharding: tensor-parallel over heads across 8 cores
(core c: Q heads 4c..4c+3, KV head c). Each core computes a partial
output (its heads through wo rows); host sums the 8 bf16 partials in
fp32 and casts to bf16.

Layout strategy (per core, per batch):
  - All matmul operands bf16; accumulation fp32 in PSUM.
  - Projections computed transposed: QKV^T[384, S] = wqkv^T @ x^T so that
    Q^T/K^T (head-dim on partitions) feed the scores matmul directly.
  - RoPE: even/odd pair interleave is folded into wq/wk/wo columns on the
    host (perm = evens-then-odds), turning the pair swap into a 32-row
    block swap done with a small permutation matmul on PE.
  - Scores computed transposed per (b,h): S^T[k,q] = K^T.T @ Q^T, so the
    softmax denominator and P@V both contract over k = partitions:
    PV lhsT = [V | ones-col] gives O^T rows 0:64 and sumexp in row 64.
  - Causal: only k-tiles <= q-tile are computed; diagonal 128x128 blocks
    get an additive triangular mask in PSUM before exp; fully-invalid
    column strips of the exp tile are memset to 0 afterwards.
  - exp on ScalarE reads PSUM strips [128, 1024] and writes bf16 SBUF.
  - Normalization: recip = 1/sumexp (DVE), broadcast across 64 partitions
    with a K=1 ones matmul, multiplied into O^T during evacuation.
"""

import sys

sys.path.insert(0, "/opt/trn_rl_repo")

import math
import numpy as np
import ml_dtypes

BF16 = ml_dtypes.bfloat16

# Problem constants (hardcoded per contract).
B = 2
S = 2048
D = 2048
N_HEADS = 32
N_KV_HEADS = 8
HD = 64
N_CORES = 8
HQ = N_HEADS // N_CORES  # 4 q heads per core
M_PROJ = HQ * HD + 2 * HD  # 384: [Q0 Q1 Q2 Q3 | K | V]
QTS = 512  # q tile size (free dim)
KTS = 128  # k tile size (partitions)
GRP = 2  # k-tiles per exp strip


def build_program(s=S, d=D, phase_log=None):
    import concourse.bass as bass
    import concourse.mybir as mybir
    import concourse.tile as tile
    from concourse import bacc

    def mark(label):
        if phase_log is not None:
            phase_log.append((label, len(nc.inst_map)))

    f32 = mybir.dt.float32
    bf16 = mybir.dt.bfloat16
    Exp = mybir.ActivationFunctionType.Exp
    Copy = mybir.ActivationFunctionType.Copy
    add_op = mybir.AluOpType.add
    mult_op = mybir.AluOpType.mult

    n_qt = s // QTS  # q tiles per batch
    n_dkt = d // 128  # contraction tiles for projections
    n_skt = s // KTS  # k tiles per batch
    n_nt = s // QTS  # token tiles (512) for proj free dim
    n_mo = (HQ * HD) // 128  # wo contraction tiles (2)

    nc = bacc.Bacc("TRN2", num_devices=N_CORES)
    xT_d = nc.declare_dram_parameter("xT", [B, d, s], bf16, isOutput=False)
    wqkv_d = nc.declare_dram_parameter("wqkv", [d, M_PROJ], bf16, isOutput=False)
    wo_d = nc.declare_dram_parameter("wo_s", [HQ * HD, d], bf16, isOutput=False)
    cos_d = nc.declare_dram_parameter("cosb", [128, s], bf16, isOutput=False)
    sin_d = nc.declare_dram_parameter("sinb", [128, s], bf16, isOutput=False)
    tri_d = nc.declare_dram_parameter("trimask", [128, 128], f32, isOutput=False)
    tri01_d = nc.declare_dram_parameter("tri01", [128, 4, QTS], bf16, isOutput=False)
    part_d = nc.declare_dram_parameter("part", [B * s, d], bf16, isOutput=True)

    with tile.TileContext(nc) as tc:
        with (
            tc.tile_pool(name="const", bufs=1) as cpool,
            tc.tile_pool(name="big", bufs=1) as bpool,
            tc.tile_pool(name="work", bufs=3) as wpool,
            tc.tile_pool(name="estrip", bufs=5) as epool,
            tc.tile_pool(name="outp", bufs=4) as opool,
            tc.tile_pool(name="psw", bufs=3, space="PSUM") as psw,
            tc.tile_pool(name="pssc", bufs=2, space="PSUM") as pssc,
            tc.tile_pool(name="psops", bufs=1, space="PSUM") as psops,
        ):
            # ---- constants / weights ----
            cos_sb = cpool.tile([128, s], bf16)
            sin_sb = cpool.tile([128, s], bf16)
            tri_sb = cpool.tile([128, 128], f32)
            tri01_sb = cpool.tile([128, 4, QTS], bf16)
            ones_sb = cpool.tile([1, 64], f32)
            wqkv_sb = cpool.tile([128, n_dkt, M_PROJ], bf16)
            wo_sb = cpool.tile([128, n_mo, d], bf16)

            nc.sync.dma_start(cos_sb[:], cos_d[:])
            nc.sync.dma_start(sin_sb[:], sin_d[:])
            nc.sync.dma_start(tri_sb[:], tri_d[:])
            nc.sync.dma_start(tri01_sb[:], tri01_d[:])
            nc.gpsimd.memset(ones_sb[:], 1.0)
            for kt in range(n_dkt):
                nc.sync.dma_start(
                    wqkv_sb[:, kt, :], wqkv_d[kt * 128 : (kt + 1) * 128, :]
                )
            for kt in range(n_mo):
                nc.sync.dma_start(wo_sb[:, kt, :], wo_d[kt * 128 : (kt + 1) * 128, :])

            # ---- per-batch persistent tiles ----
            tiles = {}

            def load_x(b):
                xT_sb = bpool.tile([128, n_dkt, s], bf16, tag="xT")
                tiles[("x", b)] = xT_sb
                for kt in range(n_dkt):
                    nc.sync.dma_start(
                        xT_sb[:, kt, :],
                        xT_d[b, kt * 128 : (kt + 1) * 128, :],
                    )

            def proj(b):
                mark(f"b{b}_proj")
                xT_sb = tiles[("x", b)]
                QT_sb = bpool.tile([128, n_mo, s], bf16, tag="QT")
                KT2_sb = bpool.tile([128, s], bf16, tag="KT2")
                VT_sb = bpool.tile([128, s], bf16, tag="VT")
                V_sb = bpool.tile([128, n_skt, 128], bf16, tag="V")
                tiles[("QT", b)] = QT_sb
                tiles[("KT2", b)] = KT2_sb
                tiles[("V", b)] = V_sb
                # ones column / zero pad for PV lhsT
                nc.gpsimd.memset(V_sb[:, :, 64:128], 0.0)
                nc.gpsimd.memset(V_sb[:, :, 64:65], 1.0)
                for m in (2, 0, 1):  # K/V first so attention can start early
                    for n in range(n_nt):
                        nsl = slice(n * QTS, (n + 1) * QTS)
                        ps = psw.tile([128, QTS], f32, tag="w")
                        for kt in range(n_dkt):
                            nc.tensor.matmul(
                                ps[:],
                                wqkv_sb[:, kt, m * 128 : (m + 1) * 128],
                                xT_sb[:, kt, nsl],
                                start=(kt == 0),
                                stop=(kt == n_dkt - 1),
                            )
                        if m < 2:
                            # two Q heads stacked: rope all 128 rows
                            q_raw = wpool.tile([128, QTS], bf16, tag="qraw")
                            nc.scalar.activation(q_raw[:], ps[:], Copy)
                            t1 = wpool.tile([128, QTS], bf16, tag="t1")
                            t2 = wpool.tile([128, QTS], bf16, tag="t2")
                            nc.vector.tensor_tensor(
                                t1[:], q_raw[:], cos_sb[:, nsl], mult_op
                            )
                            # swap(q_raw) via cross-base copies, then * sin
                            qsw = wpool.tile([128, QTS], bf16, tag="qsw")
                            for r0, r1 in ((0, 32), (32, 0), (64, 96), (96, 64)):
                                nc.vector.tensor_copy(
                                    qsw[r0 : r0 + 32, :], q_raw[r1 : r1 + 32, :]
                                )
                            nc.vector.tensor_tensor(
                                t2[:], qsw[:], sin_sb[:, nsl], mult_op
                            )
                            nc.vector.tensor_tensor(
                                QT_sb[:, m, nsl], t1[:], t2[:], add_op
                            )
                        else:
                            # rows 0:64 = K^T (rope), rows 64:128 = V^T (copy)
                            q_raw = wpool.tile([128, QTS], bf16, tag="qraw")
                            nc.scalar.activation(q_raw[0:64, :], ps[0:64, :], Copy)
                            t1 = wpool.tile([128, QTS], bf16, tag="t1")
                            t2 = wpool.tile([128, QTS], bf16, tag="t2")
                            nc.vector.tensor_tensor(
                                t1[0:64, :], q_raw[0:64, :], cos_sb[0:64, nsl], mult_op
                            )
                            qsw = wpool.tile([128, QTS], bf16, tag="qsw")
                            for r0, r1 in ((0, 32), (32, 0)):
                                nc.vector.tensor_copy(
                                    qsw[r0 : r0 + 32, :], q_raw[r1 : r1 + 32, :]
                                )
                            nc.vector.tensor_tensor(
                                t2[0:64, :], qsw[0:64, :], sin_sb[0:64, nsl], mult_op
                            )
                            nc.vector.tensor_tensor(
                                KT2_sb[0:64, nsl], t1[0:64, :], t2[0:64, :], add_op
                            )
                            # duplicate K^T into partitions 64:128 (row-group packing)
                            nc.vector.tensor_copy(
                                KT2_sb[64:128, nsl], KT2_sb[0:64, nsl]
                            )
                            # V^T: plain cast copy into partitions 64:128
                            nc.scalar.activation(
                                VT_sb[64:128, nsl], ps[64:128, :], Copy
                            )
                    if m == 2:
                        # V^T -> V (token-major) via DMA transpose
                        mark(f"b{b}_vtr")
                        for kt in range(n_skt):
                            nc.sync.dma_start_transpose(
                                V_sb[:, kt, 0:64],
                                VT_sb[64:128, kt * KTS : (kt + 1) * KTS],
                            )
                        mark(f"b{b}_proj2")

            def attn(b):
                mark(f"b{b}_attn")
                QT_sb = tiles[("QT", b)]
                KT2_sb = tiles[("KT2", b)]
                V_sb = tiles[("V", b)]
                OT_sb = bpool.tile([128, n_mo, s], bf16, tag="OT")
                tiles[("OT", b)] = OT_sb
                pending = []

                def normalize(hb2, m2, qsl2, ops2, rt2):
                    # recip already issued (DVE); broadcast + scale into OT
                    bps = psw.tile([128, QTS], f32, tag="w")
                    nc.tensor.matmul(
                        bps[0:64, :], ones_sb[:], rt2[:], start=True, stop=True
                    )
                    bsb = wpool.tile([64, QTS], f32, tag="bsb")
                    nc.any.tensor_copy(bsb[:], bps[0:64, :])
                    nc.vector.tensor_tensor(
                        OT_sb[hb2 : hb2 + 64, m2, qsl2],
                        ops2[0:64, :],
                        bsb[:],
                        mult_op,
                    )

                for qt in range(n_qt):
                    for h in range(HQ):
                        hb = (h % 2) * 64
                        qh = QT_sb[hb : hb + 64, h // 2, :]
                        kt2 = KT2_sb[hb : hb + 64, :]
                        qsl = slice(qt * QTS, (qt + 1) * QTS)
                        n_kt = (qt + 1) * (QTS // KTS)  # k tiles needed
                        ops = psops.tile([128, QTS], f32, tag="ops")
                        for g in range(0, n_kt, GRP):
                            kts = list(range(g, min(g + GRP, n_kt)))
                            sc = pssc.tile([128, GRP * QTS], f32, tag="sc")
                            e = epool.tile([128, GRP * QTS], bf16, tag="e")
                            for j, kt in enumerate(kts):
                                nc.tensor.matmul(
                                    sc[:, j * QTS : (j + 1) * QTS],
                                    kt2[:, kt * KTS : (kt + 1) * KTS],
                                    qh[:, qsl],
                                    start=True,
                                    stop=True,
                                )
                            if g == 0 and pending:
                                # normalize the previous q-tile now; its recip
                                # had time to finish, so PE doesn't stall
                                normalize(*pending.pop())
                            nc.scalar.activation(
                                e[:, 0 : len(kts) * QTS], sc[:, 0 : len(kts) * QTS], Exp
                            )
                            for j, kt in enumerate(kts):
                                o = kt * KTS - qt * QTS
                                if o >= 0:  # diagonal tile: 0/1 mask after exp
                                    nc.vector.tensor_tensor(
                                        e[:, j * QTS : (j + 1) * QTS],
                                        e[:, j * QTS : (j + 1) * QTS],
                                        tri01_sb[:, o // KTS, :],
                                        mult_op,
                                    )
                                nc.tensor.matmul(
                                    ops[:],
                                    V_sb[:, kt, :],
                                    e[:, j * QTS : (j + 1) * QTS],
                                    start=(kt == 0),
                                    stop=(kt == n_kt - 1),
                                )
                        rt = wpool.tile([1, QTS], f32, tag="rt")
                        nc.vector.reciprocal(rt[:], ops[64:65, :])
                        pending.append((hb, h // 2, qsl, ops, rt))
                    if qt > 0:
                        wo_block(b, qt - 1)
                if pending:
                    normalize(*pending.pop())
                wo_block(b, n_qt - 1)

            def wo_block(b, qt):
                OT_sb = tiles[("OT", b)]
                for mt in range(4 * qt, 4 * qt + 4):
                    msl = slice(mt * 128, (mt + 1) * 128)
                    osb = opool.tile([128, d], bf16, tag="osb")
                    for n in range(d // QTS):
                        nsl = slice(n * QTS, (n + 1) * QTS)
                        pool = psw if n % 2 == 0 else pssc
                        ps = pool.tile([128, QTS], f32, tag="w" if n % 2 == 0 else "sc")
                        for kt in range(n_mo):
                            nc.tensor.matmul(
                                ps[:],
                                OT_sb[:, kt, msl],
                                wo_sb[:, kt, nsl],
                                start=(kt == 0),
                                stop=(kt == n_mo - 1),
                            )
                        nc.any.tensor_copy(osb[:, nsl], ps[:])
                    nc.sync.dma_start(
                        part_d[b * s + mt * 128 : b * s + (mt + 1) * 128, :],
                        osb[:],
                    )

            load_x(0)
            proj(0)
            load_x(1)  # b1 input load overlaps b0 attention (SP order)
            attn(0)
            proj(1)
            attn(1)
    mark("end")
    nc.compile()
    return nc


# ---------------- host-side sharding ----------------

_PERM = np.concatenate([np.arange(0, HD, 2), np.arange(1, HD, 2)])  # evens, odds


def make_core_inputs(x, freqs_cos, freqs_sin, wq, wk, wv, wo, s=S, d=D):
    """Build per-core input maps (list of dicts, one per core)."""
    b = x.shape[0]
    xT = np.ascontiguousarray(np.transpose(x, (0, 2, 1))).astype(BF16)  # [B, D, S]

    cosT = np.ascontiguousarray(freqs_cos.T)  # [32, S]
    sinT = np.ascontiguousarray(freqs_sin.T)
    cosb = np.tile(np.concatenate([cosT, cosT], axis=0), (2, 1)).astype(BF16)  # [128,S]
    sinb = np.tile(
        np.concatenate([-sinT, sinT], axis=0), (2, 1)
    ).astype(BF16)

    p = np.arange(128)[:, None]
    f = np.arange(128)[None, :]
    trimask = np.where(f >= p, 0.0, -1e9).astype(np.float32)
    f5 = np.arange(QTS)[None, :]
    tri01 = np.stack(
        [np.where(f5 >= o + p, 1.0, 0.0) for o in (0, 128, 256, 384)], axis=1
    ).astype(BF16)  # [128, 4, 512]

    scale = 1.0 / math.sqrt(HD)
    in_maps = []
    for c in range(N_CORES):
        wq_c = np.concatenate(
            [
                wq[:, (4 * c + h) * HD : (4 * c + h + 1) * HD][:, _PERM]
                for h in range(HQ)
            ],
            axis=1,
        ) * scale
        wk_c = wk[:, c * HD : (c + 1) * HD][:, _PERM]
        wv_c = wv[:, c * HD : (c + 1) * HD]
        wqkv = np.concatenate([wq_c, wk_c, wv_c], axis=1).astype(BF16)  # [D, 384]
        wo_c = np.ascontiguousarray(
            wo[4 * c * HD : (4 * c + HQ) * HD, :]
        ).astype(BF16)  # [256, D] — O is in original d-order (V unpermuted)
        in_maps.append(
            {
                "xT": xT,
                "wqkv": wqkv,
                "wo_s": wo_c,
                "cosb": cosb,
                "sinb": sinb,
                "trimask": trimask,
                "tri01": tri01,
            }
        )
    return in_maps


_NC_CACHE = {}


def kernel(x, freqs_cos, freqs_sin, wq, wk, wv, wo):
    from concourse.bass_utils import run_bass_kernel_spmd

    x = np.asarray(x, np.float32)
    freqs_cos = np.asarray(freqs_cos, np.float32)
    freqs_sin = np.asarray(freqs_sin, np.float32)
    wq = np.asarray(wq, np.float32)
    wk = np.asarray(wk, np.float32)
    wv = np.asarray(wv, np.float32)
    wo = np.asarray(wo, np.float32)

    if "nc" not in _NC_CACHE:
        _NC_CACHE["nc"] = build_program()
    nc = _NC_CACHE["nc"]

    in_maps = make_core_inputs(x, freqs_cos, freqs_sin, wq, wk, wv, wo)
    res = run_bass_kernel_spmd(nc, in_maps, list(range(N_CORES)))
    acc = np.zeros((B * S, D), np.float32)
    for r in res.results:
        acc += np.asarray(r["part"], np.float32)
    return acc.reshape(B, S, D).astype(BF16)



# revision 20
# speedup vs baseline: 1.4001x; 1.4001x over previous
"""Trainium2 Bass kernel for nn_Attention_78151224918608.

Dense transformer attention block: QKV proj + RoPE + GQA causal attention
+ output proj. Sharding: 2 batches x 4 head-groups across 8 cores
(core c: batch c//4, Q heads 8g..8g+7, KV heads 2g..2g+1 for g=c%4).
Each core computes a partial output through its wo rows; host sums the
4 partials per batch in fp32 and casts to bf16.

Key structure (per core):
  - Projections QKV^T = wqkv^T @ x^T with fp8e4m3 DoubleRow matmuls using
    a 3-term compensated split (hi@hi + lo@hi + hi@lo, lo = residual) that
    keeps ~bf16 accuracy at 0.375x the bf16 PE cycles. Weights are
    host-scaled by powers of 2 into e4m3's normal range; inverse scales
    are folded into the rope cos/sin tables (Q, K), the V evacuation, and
    the host gather (wo).
  - x^T streamed per 512-token block; attention for q-tile i is emitted
    right after projection block i, so exp (ACT) overlaps later proj.
  - RoPE pair-deinterleave folded into weight column order; the remaining
    32-row block swap + cos/sin multiplies on DVE.
  - Scores per (head, q-tile): S^T[k,q] = K^T.T @ Q^T into PSUM strips of
    2 k-tiles, causally trimmed columns; exp on ACT writes bf16 SBUF; 0/1
    triangle mask multiplied into diagonal chunks on DVE.
  - PV transposed: O'[q,hd] = e_chunk.T @ [V|1] with the e chunk as the
    stationary operand -> 65 PE cycles per chain step; sumexp lands in
    column 64 as a per-partition scalar.
  - Normalize: reciprocal + per-partition tensor_scalar multiply (DVE),
    then one batched DMA-transpose per (head, q-tile) into OT[hd, tok]
    (token dim padded to 132 per 128-block so the transpose AP stays 3D).
  - Output proj: OT split hi/lo fp8 on device (Pool copy + DVE subtract);
    3-term compensated DoubleRow against host-split wo; partials stored
    bf16 at x64 scale, divided out in the host gather.
"""

import sys

sys.path.insert(0, "/opt/trn_rl_repo")

import math
import numpy as np
import ml_dtypes

BF16 = ml_dtypes.bfloat16
E4M3 = ml_dtypes.float8_e4m3

# Problem constants (hardcoded per contract).
B = 2
S = 2048
D = 2048
N_HEADS = 32
N_KV_HEADS = 8
HD = 64
N_CORES = 8
N_GRP = 4  # head groups
HQ = N_HEADS // N_GRP  # 8 q heads per core
HKV = N_KV_HEADS // N_GRP  # 2 kv heads per core
M_PROJ = HQ * HD + 2 * HKV * HD  # 768 cols: [Q pairs x4 | K | V]
QTS = 512
KTS = 128
XQ_SCALE = 512.0  # wq_eff = wq/8, scaled into e4m3 normal range
K_SCALE = 64.0  # wk, wv, wo scale




def build_program(s=S, d=D, phase_log=None, debug=False):
    import concourse.bass as bass
    import concourse.mybir as mybir
    import concourse.tile as tile
    from concourse import bacc

    def mark(label):
        if phase_log is not None:
            phase_log.append((label, len(nc.inst_map)))

    f32 = mybir.dt.float32
    bf16 = mybir.dt.bfloat16
    fp8 = mybir.dt.float8e4
    Exp = mybir.ActivationFunctionType.Exp
    Copy = mybir.ActivationFunctionType.Copy
    DR = mybir.MatmulPerfMode.DoubleRow
    add_op = mybir.AluOpType.add
    sub_op = mybir.AluOpType.subtract
    mult_op = mybir.AluOpType.mult

    n_dp = d // 256  # 8 DoubleRow pairs over the d contraction
    n_nt = s // QTS  # 4 token blocks
    n_qt = s // QTS  # 4 q tiles
    n_skt = s // KTS  # 16 k tiles
    n_tt = s // KTS  # 16 token tiles for wo

    nc = bacc.Bacc("TRN2", num_devices=N_CORES)
    xh_d = nc.declare_dram_parameter("xh", [128, n_dp, 2, s], fp8, isOutput=False)
    xl_d = nc.declare_dram_parameter("xl", [128, n_dp, 2, s], fp8, isOutput=False)
    wh_d = nc.declare_dram_parameter("wh", [128, n_dp, 2, M_PROJ], fp8, isOutput=False)
    wl_d = nc.declare_dram_parameter("wl", [128, n_dp, 2, M_PROJ], fp8, isOutput=False)
    woh_d = nc.declare_dram_parameter("woh", [128, 2, 2, d], fp8, isOutput=False)
    wol_d = nc.declare_dram_parameter("wol", [128, 2, 2, d], fp8, isOutput=False)
    cosq_d = nc.declare_dram_parameter("cosq", [128, s], bf16, isOutput=False)
    sinq_d = nc.declare_dram_parameter("sinq", [128, s], bf16, isOutput=False)
    cosk_d = nc.declare_dram_parameter("cosk", [128, s], bf16, isOutput=False)
    sink_d = nc.declare_dram_parameter("sink", [128, s], bf16, isOutput=False)
    tri01_d = nc.declare_dram_parameter("tri01", [128, 128], bf16, isOutput=False)
    part_d = nc.declare_dram_parameter("part", [s, d], bf16, isOutput=True)
    if debug:
        dbg = {
            "dQT": nc.declare_dram_parameter("dQT", [128, 4, s], bf16, isOutput=True),
            "dKT": nc.declare_dram_parameter("dKT", [128, s], bf16, isOutput=True),
            "dVT": nc.declare_dram_parameter("dVT", [128, s], bf16, isOutput=True),
            "dV": nc.declare_dram_parameter("dV", [128, HKV, 16, 128], bf16, isOutput=True),
            "dOT": nc.declare_dram_parameter("dOT", [128, 4, s], bf16, isOutput=True),
            "dE": nc.declare_dram_parameter("dE", [128, 2, 512], bf16, isOutput=True),
            "dON": nc.declare_dram_parameter("dON", [128, 4, 128], bf16, isOutput=True),
        }

    with tile.TileContext(nc) as tc:
        with (
            tc.tile_pool(name="const", bufs=1) as cpool,
            tc.tile_pool(name="xstr", bufs=2) as xpool,
            tc.tile_pool(name="big", bufs=1) as bpool,
            tc.tile_pool(name="work", bufs=2) as wpool,
            tc.tile_pool(name="estrip", bufs=10) as epool,
            tc.tile_pool(name="onrm", bufs=3) as opool,
            tc.tile_pool(name="outp", bufs=2) as spool,
            tc.tile_pool(name="psw", bufs=2, space="PSUM") as psw,
            tc.tile_pool(name="pssc", bufs=2, space="PSUM") as pssc,
            tc.tile_pool(name="psops", bufs=2, space="PSUM") as psops,
        ):
            # ---- constants / weights ----
            cosq_sb = cpool.tile([128, s], bf16)
            sinq_sb = cpool.tile([128, s], bf16)
            cosk_sb = cpool.tile([128, s], bf16)
            sink_sb = cpool.tile([128, s], bf16)
            tri01_sb = cpool.tile([128, 128], bf16)
            wh_sb = cpool.tile([128, n_dp, 2, M_PROJ], fp8)
            wl_sb = cpool.tile([128, n_dp, 2, M_PROJ], fp8)
            woh_sb = cpool.tile([128, 2, 2, d], fp8)
            wol_sb = cpool.tile([128, 2, 2, d], fp8)

            for sb, dr in (
                (wh_sb, wh_d),
                (wl_sb, wl_d),
                (woh_sb, woh_d),
                (wol_sb, wol_d),
                (cosq_sb, cosq_d),
                (sinq_sb, sinq_d),
                (cosk_sb, cosk_d),
                (sink_sb, sink_d),
                (tri01_sb, tri01_d),
            ):
                nc.sync.dma_start(sb[:], dr[:])

            # ---- persistent per-core tensors ----
            QT_sb = bpool.tile([128, 4, s], bf16)  # m-tile m: heads (m, m+4)
            KT_sb = bpool.tile([128, s], bf16)  # 2 kv heads stacked
            VT_sb = bpool.tile([128, s], bf16)
            V_sb = bpool.tile([128, HKV, n_skt, 128], bf16)
            OT_sb = bpool.tile([128, 4, s], bf16)
            OTh_sb = bpool.tile([128, 4, s], fp8)
            OTl_sb = bpool.tile([128, 4, s], fp8)
            nc.gpsimd.memset(V_sb[:, :, :, 64:128], 0.0)
            nc.gpsimd.memset(V_sb[:, :, :, 64:65], 1.0)

            # ---------------- projection block n ----------------
            def proj_block(n):
                mark(f"proj{n}")
                nsl = slice(n * QTS, (n + 1) * QTS)
                xh_sb = xpool.tile([128, n_dp, 2, QTS], fp8, tag="xh")
                xl_sb = xpool.tile([128, n_dp, 2, QTS], fp8, tag="xl")
                nc.sync.dma_start(xh_sb[:], xh_d[:, :, :, nsl])
                nc.sync.dma_start(xl_sb[:], xl_d[:, :, :, nsl])
                for m in (5, 4, 0, 1, 2, 3):
                    msl = slice(m * 128, (m + 1) * 128)
                    ps = psw.tile([128, QTS], f32, tag="w")
                    terms = ((wh_sb, xh_sb), (wl_sb, xh_sb), (wh_sb, xl_sb))
                    for ti, (wsb, xsb) in enumerate(terms):
                        for dp in range(n_dp):
                            nc.tensor.matmul(
                                ps[:],
                                wsb[:, dp, :, msl],
                                xsb[:, dp, :, :],
                                start=(ti == 0 and dp == 0),
                                stop=(ti == 2 and dp == n_dp - 1),
                                perf_mode=DR,
                            )
                    if m == 5:
                        nc.scalar.activation(
                            VT_sb[:, nsl], ps[:], Copy, scale=1.0 / K_SCALE
                        )
                        continue
                    q_raw = wpool.tile([128, QTS], bf16, tag="qraw", bufs=3)
                    nc.scalar.activation(q_raw[:], ps[:], Copy)
                    # rope (scales folded in cos/sin tables)
                    cos_sb = cosk_sb if m == 4 else cosq_sb
                    sin_sb = sink_sb if m == 4 else sinq_sb
                    dst = KT_sb[:, nsl] if m == 4 else QT_sb[:, m, nsl]
                    t1 = wpool.tile([128, QTS], bf16, tag="t1")
                    qsw = wpool.tile([128, QTS], bf16, tag="qsw")
                    nc.vector.tensor_tensor(t1[:], q_raw[:], cos_sb[:, nsl], mult_op)
                    for r0, r1 in ((0, 32), (32, 0), (64, 96), (96, 64)):
                        nc.vector.tensor_copy(
                            qsw[r0 : r0 + 32, :], q_raw[r1 : r1 + 32, :]
                        )
                    nc.vector.tensor_tensor(qsw[:], qsw[:], sin_sb[:, nsl], mult_op)
                    nc.vector.tensor_tensor(dst, t1[:], qsw[:], add_op)
                # V k-tiles of this block become usable once transposed
                # (per-tile 2D transposes: multi-chunk out APs with padded
                # strides are not supported by the HW descriptor layout)
                for kv in range(HKV):
                    for kt in range(4 * n, 4 * n + 4):
                        nc.sync.dma_start_transpose(
                            V_sb[:, kv, kt, 0:64],
                            VT_sb[kv * 64 : (kv + 1) * 64, kt * KTS : (kt + 1) * KTS],
                        )

            # ---------------- attention ----------------
            def attn_head_qt(h, qt, on):
                hb = (h // 4) * 64  # kv partition half
                kv = h // 4
                qm = h % 4
                qbase = qt * QTS
                n_kt = (qt + 1) * (QTS // KTS)  # valid k tiles
                strips = []
                for g in range(n_kt // 2):
                    off = max(0, 256 * g - 512 * qt)
                    sc = pssc.tile([128, 2, QTS], f32, tag="sc")
                    e = epool.tile([128, 2, QTS], bf16, tag="e")
                    for j in (0, 1):
                        kt = 2 * g + j
                        nc.tensor.matmul(
                            sc[:, j, off:QTS],
                            KT_sb[hb : hb + 64, kt * KTS : (kt + 1) * KTS],
                            QT_sb[hb : hb + 64, qm, qbase + off : qbase + QTS],
                            start=True,
                            stop=True,
                        )
                    nc.scalar.activation(e[:, :, off:QTS], sc[:, :, off:QTS], Exp)
                    # 0/1 mask on diagonal chunks (k-tile kt == global q row)
                    for j in (0, 1):
                        kt = 2 * g + j
                        c0 = kt * KTS - qbase
                        if 0 <= c0 < QTS:
                            nc.gpsimd.tensor_tensor(
                                e[:, j, c0 : c0 + 128],
                                e[:, j, c0 : c0 + 128],
                                tri01_sb[:],
                                mult_op,
                            )
                    if debug and h == 0 and qt == 0 and g == 0:
                        nc.sync.dma_start(dbg["dE"][:], e[:])
                    strips.append(e)

                ob = (h % 2) * HD  # column half of the paired transpose input
                for r in range(4):
                    qr = 4 * qt + r  # global q row
                    ops = psops.tile([128, 72], f32, tag="ops")
                    for kt in range(qr + 1):
                        e = strips[kt // 2]
                        nc.tensor.matmul(
                            ops[:, 0:65],
                            e[:, kt % 2, r * 128 : (r + 1) * 128],
                            V_sb[:, kv, kt, 0:65],
                            start=(kt == 0),
                            stop=(kt == qr),
                        )
                    rt = wpool.tile([128, 1], f32, tag="rt", bufs=4)
                    nc.vector.reciprocal(rt[:], ops[:, 64:65])
                    nc.vector.tensor_scalar(
                        on[:, r, ob : ob + HD], ops[:, 0:64], rt[:], None, op0=mult_op
                    )
                if debug and h == 1 and qt == 0:
                    nc.sync.dma_start(dbg["dON"][:], on[:])
                if h % 2 == 1:
                    # paired transpose: on[q, r, 0:64|64:128] holds heads
                    # (h-1, h); each 128-col input block r is transposed into
                    # the 128-col out chunk selected by the out AP's middle
                    # dim (explicit 3D; out partition base must be 0)
                    out3 = OT_sb[:, h // 2, qbase : qbase + QTS].rearrange(
                        "p (r q) -> p r q", r=4
                    )
                    nc.sync.dma_start_transpose(out3, on[:])

            # ---------------- OT hi/lo split + wo ----------------
            def ot_split(qt):
                tsl = slice(qt * QTS, (qt + 1) * QTS)
                for mt in range(4):
                    nc.gpsimd.tensor_copy(OTh_sb[:, mt, tsl], OT_sb[:, mt, tsl])
                    nc.gpsimd.tensor_tensor(
                        OTl_sb[:, mt, tsl],
                        OT_sb[:, mt, tsl],
                        OTh_sb[:, mt, tsl],
                        sub_op,
                    )

            def wo_block(qt):
                for tt in range(4 * qt, 4 * qt + 4):
                    msl = slice(tt * 128, (tt + 1) * 128)
                    osb = spool.tile([128, d], bf16, tag="osb")
                    for n in range(d // QTS):
                        nsl = slice(n * QTS, (n + 1) * QTS)
                        ps = psw.tile([128, QTS], f32, tag="w")
                        terms = ((OTh_sb, woh_sb), (OTl_sb, woh_sb), (OTh_sb, wol_sb))
                        for ti, (osrc, wsrc) in enumerate(terms):
                            for pr in range(2):
                                nc.tensor.matmul(
                                    ps[:],
                                    osrc[:, 2 * pr : 2 * pr + 2, msl],
                                    wsrc[:, pr, :, nsl],
                                    start=(ti == 0 and pr == 0),
                                    stop=(ti == 2 and pr == 1),
                                    perf_mode=DR,
                                )
                        nc.vector.tensor_copy(osb[:, nsl], ps[:])
                    nc.sync.dma_start(part_d[msl, :], osb[:])

            for qt in range(n_qt):
                proj_block(qt)
                mark(f"attn{qt}")
                on = None
                for h in range(HQ):
                    if h % 2 == 0:
                        on = opool.tile([128, 4, 128], bf16, tag="on")
                    attn_head_qt(h, qt, on)
                ot_split(qt)
                if qt > 0:
                    wo_block(qt - 1)
            wo_block(n_qt - 1)
            if debug:
                nc.sync.dma_start(dbg["dQT"][:], QT_sb[:])
                nc.sync.dma_start(dbg["dKT"][:], KT_sb[:])
                nc.sync.dma_start(dbg["dVT"][:], VT_sb[:])
                nc.sync.dma_start(dbg["dV"][:], V_sb[:])
                nc.sync.dma_start(dbg["dOT"][:], OT_sb[:])
    mark("end")
    nc.compile()
    return nc


# ---------------- host-side prep ----------------

_PERM = np.concatenate([np.arange(0, HD, 2), np.arange(1, HD, 2)])  # evens, odds


def _split_fp8(a):
    hi = a.astype(E4M3)
    lo = (a - hi.astype(np.float32)).astype(E4M3)
    return hi, lo


def _pair_layout(w, n_pair):
    """[DD, M] -> [128, n_pair, 2, M] with contraction d = dp*256 + j*128 + p."""
    dd, m = w.shape
    assert dd == n_pair * 256
    return np.ascontiguousarray(w.reshape(n_pair, 2, 128, m).transpose(2, 0, 1, 3))


def make_core_inputs(x, freqs_cos, freqs_sin, wq, wk, wv, wo, s=S, d=D):
    n_dp = d // 256

    cosT = np.ascontiguousarray(freqs_cos.T).astype(np.float32)  # [32, S]
    sinT = np.ascontiguousarray(freqs_sin.T).astype(np.float32)

    def tables(scale):
        cosb = np.tile(np.concatenate([cosT, cosT], axis=0), (2, 1)) * scale
        sinb = np.tile(np.concatenate([-sinT, sinT], axis=0), (2, 1)) * scale
        return cosb.astype(BF16), sinb.astype(BF16)

    cosq, sinq = tables(1.0 / XQ_SCALE)
    cosk, sink = tables(1.0 / K_SCALE)

    p = np.arange(128)[:, None]
    f = np.arange(128)[None, :]
    tri01 = np.where(f >= p, 1.0, 0.0).astype(BF16)  # e[k, q] valid iff q >= k

    scale = 1.0 / math.sqrt(HD)
    in_maps = []
    for c in range(N_CORES):
        bb, g = c // 4, c % 4
        xT = np.ascontiguousarray(x[bb].T).astype(np.float32)  # [D, S]
        xh, xl = _split_fp8(_pair_layout(xT, n_dp))

        # wqkv cols: 4 Q m-tiles pairing heads (m, m+4), then K, V (2 kv each)
        qcols = []
        for m in range(4):
            for hh in (m, m + 4):
                hglob = g * HQ + hh
                qcols.append(
                    wq[:, hglob * HD : (hglob + 1) * HD][:, _PERM] * (scale * XQ_SCALE)
                )
        kcols = [
            wk[:, (g * HKV + kv) * HD : (g * HKV + kv + 1) * HD][:, _PERM] * K_SCALE
            for kv in range(HKV)
        ]
        vcols = [
            wv[:, (g * HKV + kv) * HD : (g * HKV + kv + 1) * HD] * K_SCALE
            for kv in range(HKV)
        ]
        wqkv = np.concatenate(qcols + kcols + vcols, axis=1)  # [D, 768]
        wh, wl = _split_fp8(_pair_layout(wqkv, n_dp))

        # wo rows for this group; row d = mt*128 + (h%2)*64 + hd matches the
        # OT layout (m-tile mt holds heads 2mt, 2mt+1)
        wo_rows = wo[g * HQ * HD : (g + 1) * HQ * HD, :] * K_SCALE  # [512, D]
        woh, wol = _split_fp8(_pair_layout(wo_rows, 2))

        in_maps.append(
            {
                "xh": xh,
                "xl": xl,
                "wh": wh,
                "wl": wl,
                "woh": woh,
                "wol": wol,
                "cosq": cosq,
                "sinq": sinq,
                "cosk": cosk,
                "sink": sink,
                "tri01": tri01,
            }
        )
    return in_maps


_NC_CACHE = {}


def kernel(x, freqs_cos, freqs_sin, wq, wk, wv, wo):
    from concourse.bass_utils import run_bass_kernel_spmd

    x = np.asarray(x, np.float32)
    freqs_cos = np.asarray(freqs_cos, np.float32)
    freqs_sin = np.asarray(freqs_sin, np.float32)
    wq = np.asarray(wq, np.float32)
    wk = np.asarray(wk, np.float32)
    wv = np.asarray(wv, np.float32)
    wo = np.asarray(wo, np.float32)

    if "nc" not in _NC_CACHE:
        _NC_CACHE["nc"] = build_program()
    nc = _NC_CACHE["nc"]

    in_maps = make_core_inputs(x, freqs_cos, freqs_sin, wq, wk, wv, wo)
    res = run_bass_kernel_spmd(nc, in_maps, list(range(N_CORES)))
    acc = np.zeros((B, S, D), np.float32)
    for c, r in enumerate(res.results):
        acc[c // 4] += np.asarray(r["part"], np.float32)
    return (acc / K_SCALE).astype(BF16)
